# revision 3
# baseline (speedup 1.0000x reference)
"""Causal self-attention (B=4, T=2048, C=1024, 16 heads x d=64) on 8 trn2 NeuronCores.

Strategy: tensor-parallel over heads — core i owns heads (2i, 2i+1).
v2: engineered to keep the PE continuously busy at the full 2.4 GHz p-state
(TRN2 drops to 1.2 GHz whenever the tensor engine stalls, and only ramps back
after ~3us of continuous execution — the v1 trace showed nearly every matmul
running at the mid/low p-state):
  - attention k-loop is software-pipelined with depth 2: the P^T V matmuls of
    block i run during block i+2, so the PE never waits on the scalar-engine
    exp (or the gpsimd causal select on diagonal blocks)
  - the normalize+projection of batch b-1 is woven into batch b's attention
    loop as PE filler (1 matmul per block) covering the exp-rate deficit
  - batch 3 bounces its qc0-2 softmax denominators early so its own proj
    chunks 0-2 weave into its qc3 k-loop, shortening the tail
  - V-transpose copies + half the y drains run on gpsimd, so PSUM recycling
    never waits on the DVE
  - x / W_qkv are bf16 (same 1 cyc/row PE rate, half the slab DMA bytes);
    q/k/v/attention stay fp32r
  - S matmuls are trimmed to the causally reachable columns on diagonal blocks
  - denominator DMA bounces ride the scalar DGE queue so their semaphore waits
    never head-of-line-block slab/output traffic on the sync queue
Layout is feature-major as in v1: host pre-transposes x once; per-core output
is the 128-feature partial projection out^T [C, B*T], summed on host.
"""

import sys

if "/opt/trn_rl_repo" not in sys.path:
    sys.path.insert(0, "/opt/trn_rl_repo")

import contextlib
import ctypes
import types

import numpy as np

import concourse.bass as bass
import concourse.mybir as mybir
import concourse.tile as tile
from concourse.bass_utils import run_bass_kernel_spmd

B, T, C = 4, 2048, 1024
N_HEAD, D = 16, 64
NCORES = 8
F32 = mybir.dt.float32
BF16 = mybir.dt.bfloat16

# matmul operand dtype for attention/proj tensors: "fp32r" (1 cyc/row at
# free-dim >= 256) or "fp32" (bit-exact, 4 cyc/row). x/W_qkv are always bf16.
DT_MM = "fp32r"
TRACE = False  # test.py flips this for profiled runs

_SO_PATH = "/opt/axon/libaxon_pjrt.so"


# ---------------------------------------------------------------------------
# Environment shims: (1) register the NTFF profile hook trn_boot could not
# install (image's antenv lacks axon_hooks); (2) this walrus build caps sem
# waits per instruction, but Tile's tail drain carries one wait per active
# proc — spread them over single-wait SP NOPs instead.
# ---------------------------------------------------------------------------
def _install_ntff_hook():
    if "antenv.axon_hooks" in sys.modules:
        return
    state = {"hook": None}

    def set_hook(h):
        state["hook"] = h

    def get_hook():
        return state["hook"]

    mod = types.ModuleType("antenv.axon_hooks")
    mod.set_axon_ntff_profile_hook = set_hook
    mod.get_axon_ntff_profile_hook = get_hook
    sys.modules["antenv.axon_hooks"] = mod
    import antenv

    antenv.axon_hooks = mod

    try:
        lib = ctypes.CDLL(_SO_PATH)
    except OSError:
        return
    if not hasattr(lib, "axon_start_nrt_profile"):
        return
    lib.axon_start_nrt_profile.argtypes = [
        ctypes.POINTER(ctypes.c_int64),
        ctypes.c_size_t,
    ]
    lib.axon_start_nrt_profile.restype = ctypes.c_int64
    lib.axon_stop_nrt_profile.argtypes = [ctypes.c_char_p]
    lib.axon_stop_nrt_profile.restype = ctypes.c_int64

    @contextlib.contextmanager
    def _hook_cm(output_dir, device_ids):
        import jax

        jax.devices()
        if device_ids:
            ids = (ctypes.c_int64 * len(device_ids))(*device_ids)
            rc = lib.axon_start_nrt_profile(ids, len(device_ids))
        else:
            rc = lib.axon_start_nrt_profile(None, 0)
        if rc != 0:
            raise RuntimeError(f"axon_start_nrt_profile rc={rc}")
        try:
            yield
        finally:
            n = lib.axon_stop_nrt_profile(str(output_dir).encode())
            if n < 0:
                raise RuntimeError(f"axon_stop_nrt_profile rc={n}")
            print(f"profile: {n} file(s) written to {output_dir}", file=sys.stderr)

    set_hook(_hook_cm)


def _patch_tile_tail_drain():
    from concourse.vector_clock import ScopedClock, VectorClock

    if getattr(tile.TileContext, "_drain_patch", False):
        return

    def patched(self, tick_clock, wait_clock):
        vc = tick_clock.global_clock
        n = len(vc)
        for proc in range(n):
            t = vc[proc]
            if t <= 0:
                continue
            sub = VectorClock([t if i == proc else 0 for i in range(n)])
            nop = self.nc.sync.nop(nofuse=True)
            wait_clock.add_sem_waits(nop.ins, ScopedClock({None: sub}))
        # Same tail as the original _drain_and_barrier, minus the multi-wait
        # drain — the NOP chain above already waited on every proc.
        self.nc.sync.drain()
        self.nc.all_engine_barrier()
        assert self.sems is not None
        popped = self.nc._tile_sem_poison_stack.pop()
        assert popped is self._sem_poison
        self.nc.clear_and_free_semaphores(list(self.sems.allocated().values()))
        self.nc.all_engine_barrier()

    tile.TileContext._drain_and_barrier = patched
    tile.TileContext._drain_patch = True


_install_ntff_hook()
_patch_tile_tail_drain()


def _split_waits(nc, limit=1):
    """This walrus build rejects instructions carrying more than ~2 sem waits.
    Spill excess waits onto preceding same-engine NOPs (program order on the
    issuing engine preserves the blocking semantics exactly)."""
    k = 0
    for fn in nc.m.functions:
        for bb in fn.blocks:
            new = []
            for ins in bb.instructions:
                si = ins.sync_info
                waits = list(si.on_wait) if si and si.on_wait else []
                if len(waits) > limit:
                    for w in waits[:-limit]:
                        nop = mybir.InstNoOp(name=f"I-wsplit-{k}")
                        k += 1
                        nop.engine = ins.engine
                        nop.sync_info = mybir.SyncInfo(on_wait=[w], on_update=[])
                        new.append(nop)
                    ins.sync_info = mybir.SyncInfo(
                        on_wait=waits[-limit:],
                        on_update=list(si.on_update) if si.on_update else [],
                    )
                new.append(ins)
            bb.instructions = new


def _op_dtype():
    return {
        "fp32": mybir.dt.float32,
        "fp32r": mybir.dt.float32r,
        "bf16": mybir.dt.bfloat16,
    }[DT_MM]


def build_nc():
    DT = _op_dtype()
    nc = bass.Bass()
    xT = nc.declare_dram_parameter("xT", [C, B * T], BF16, isOutput=False)
    wqkv = nc.declare_dram_parameter("wqkv", [C, 384], BF16, isOutput=False)
    bqkv = nc.declare_dram_parameter("bqkv", [128, 3], F32, isOutput=False)
    wproj = nc.declare_dram_parameter("wproj", [128, C], DT, isOutput=False)
    ident = nc.declare_dram_parameter("ident", [128, 128], DT, isOutput=False)
    outT = nc.declare_dram_parameter("outT", [C, B * T], BF16, isOutput=True)

    EXP = mybir.ActivationFunctionType.Exp

    wide = mybir.dt.size(DT) > 2  # fp32 debug dtype needs smaller pools
    with tile.TileContext(nc) as tc:
        with contextlib.ExitStack() as ctx:
            singles = ctx.enter_context(tc.tile_pool(name="singles", bufs=1))
            xpool = ctx.enter_context(tc.tile_pool(name="xpool", bufs=16))
            qkv_sb = ctx.enter_context(tc.tile_pool(name="qkv_sb", bufs=2))
            vt_pool = ctx.enter_context(tc.tile_pool(name="vtp", bufs=1))
            vaug_p = ctx.enter_context(tc.tile_pool(name="vaug", bufs=2))
            pt_pool = ctx.enter_context(tc.tile_pool(name="ptp", bufs=2 if wide else 4))
            yt_pool = ctx.enter_context(tc.tile_pool(name="ytp", bufs=1 if wide else 2))
            yub_p = ctx.enter_context(tc.tile_pool(name="yub", bufs=1 if wide else 2))
            sm_pool = ctx.enter_context(tc.tile_pool(name="smp", bufs=1))
            rb_pool = ctx.enter_context(tc.tile_pool(name="rbp", bufs=1 if wide else 2))
            ost_pool = ctx.enter_context(tc.tile_pool(name="ost", bufs=3 if wide else 6))
            dscr = ctx.enter_context(tc.tile_pool(name="dscr", bufs=2, space="DRAM"))
            # PSUM (8 banks): s [128,2,512] x2 = 4, y0+y1 [65,512] x1 = 2,
            # ps1 [128,512] x2 = 2 (qkv accum / transposes / proj out)
            ps1 = ctx.enter_context(tc.tile_pool(name="ps1", bufs=2, space="PSUM"))
            ps_s = ctx.enter_context(tc.tile_pool(name="ps_s", bufs=2, space="PSUM"))
            ps_y = ctx.enter_context(tc.tile_pool(name="ps_y", bufs=1, space="PSUM"))

            # weights ride the scalar DGE queue so the sync queue starts on
            # x slabs immediately
            wq_sb = singles.tile([128, 8, 384], BF16)
            nc.scalar.dma_start(out=wq_sb, in_=wqkv.rearrange("(a p) f -> p a f", p=128))
            wp_sb = singles.tile([128, C], DT)
            bq_sb = singles.tile([128, 3], F32)
            id_sb = singles.tile([128, 128], DT)

            def emit_late_weights():
                # bq/id are first needed ~12us in, wp only by the first
                # projection — don't let them delay the slab prefetch
                nc.scalar.dma_start(out=bq_sb, in_=bqkv[:, :])
                nc.scalar.dma_start(out=id_sb, in_=ident[:, :])
                nc.scalar.dma_start(out=wp_sb, in_=wproj[:, :])
            ones_col = singles.tile([128, 16, 1], F32)
            nc.vector.memset(ones_col, 1.0)
            # preload the Exp act table during the first qkv phase
            warm = singles.tile([1, 1], F32)
            nc.scalar.activation(warm, ones_col[0:1, 0, 0:1], EXP, scale=1.0)

            state = {}

            # ---------------- qkv phase (dense) ----------------
            def emit_slab_dmas(b, g, split_queues=False):
                st = state.setdefault(b, {"slabs": {}})
                slabs = []
                for c in range(8):
                    sl = xpool.tile([128, 1024], BF16, tag="xslab")
                    eng = nc.scalar if (split_queues and c % 2) else nc.sync
                    eng.dma_start(
                        out=sl,
                        in_=xT[
                            c * 128 : (c + 1) * 128,
                            b * T + g * 1024 : b * T + (g + 1) * 1024,
                        ],
                    )
                    slabs.append(sl)
                st["slabs"][g] = slabs

            def qkv_units(b):
                """qkv work as a list of ~0.4-1.9us PE closures so it can
                weave into the previous batch's ACT-bound attention loop:
                12 chunk units (8 accumulation matmuls + DVE bias-add) and
                8 transpose units (2 V transposes + DVE va drains)."""
                st = state.setdefault(b, {"slabs": {}})

                def u_init():
                    qT = qkv_sb.tile([128, T], DT, tag="qT")
                    kT = qkv_sb.tile([128, T], DT, tag="kT")
                    vT = vt_pool.tile([128, T], DT, tag="vT")
                    va0 = vaug_p.tile([128, 16, 65], DT, tag="va0")
                    va1 = vaug_p.tile([128, 16, 65], DT, tag="va1")
                    nc.gpsimd.tensor_copy(va0[:, :, 64:65], ones_col)
                    nc.gpsimd.tensor_copy(va1[:, :, 64:65], ones_col)
                    st.update(qT=qT, kT=kT, vT=vT, va0=va0, va1=va1)

                def chunk(g, m, half):
                    def u():
                        dst = (st["qT"], st["kT"], st["vT"])[m]
                        slabs = st["slabs"][g]
                        tch = g * 2 + half
                        ps = ps1.tile([128, 512], F32, tag="ps1")
                        for c in range(8):
                            nc.tensor.matmul(
                                ps,
                                lhsT=wq_sb[:, c, m * 128 : (m + 1) * 128],
                                rhs=slabs[c][:, half * 512 : (half + 1) * 512],
                                start=(c == 0),
                                stop=(c == 7),
                            )
                        nc.vector.tensor_scalar_add(
                            dst[:, tch * 512 : (tch + 1) * 512], ps, bq_sb[:, m : m + 1]
                        )

                    return u

                def transp2(tt0):
                    def u():
                        for tt in (tt0, tt0 + 1):
                            tp = ps1.tile([128, 128], DT, tag="ps1")
                            nc.tensor.transpose(
                                tp, st["vT"][:, tt * 128 : (tt + 1) * 128], id_sb
                            )
                            nc.vector.tensor_copy(st["va0"][:, tt, 0:64], tp[:, 0:64])
                            nc.vector.tensor_copy(st["va1"][:, tt, 0:64], tp[:, 64:128])

                    return u

                units = [u_init]
                for m in range(3):
                    for half in range(2):
                        units.append(chunk(0, m, half))
                tts = [0, 2, 4, 6]
                for m in range(3):
                    for half in range(2):
                        units.append(chunk(1, m, half))
                        if tts:
                            units.append(transp2(tts.pop(0)))
                for tt0 in (8, 10, 12, 14):
                    units.append(transp2(tt0))
                return units

            # ---------------- denominator bounce ----------------
            def emit_denom_bounce(b, r0, r1, key):
                """DMA yub partition-64 rows [r0,r1) to DRAM, back as
                [r1-r0, 512] (re-partition), fast-approx reciprocal on DVE
                (denominators are sums of positive exps — far from the
                undefined edge cases), back out to DRAM for the later
                partition-broadcast. All DMAs ride the scalar DGE queue so
                their waits never head-of-line-block slab/output traffic on
                the sync queue."""
                st = state[b]
                n = r1 - r0
                dsc = dscr.tile([1, n, 512], F32, tag=f"dsc{key}")
                nc.scalar.dma_start(out=dsc, in_=st["yub"][64:65, r0:r1, :])
                sums = sm_pool.tile([n, 512], F32, tag=f"sums{key}")
                nc.scalar.dma_start(out=sums, in_=dsc.rearrange("o h q -> (o h) q"))
                # 1/x = exp(-ln x) on the ACT engine: Ln/Exp/Copy share one
                # act table set, so no table reloads and no DVE coupling
                lns = sm_pool.tile([n, 512], F32, tag=f"lns{key}")
                nc.scalar.activation(lns, sums, mybir.ActivationFunctionType.Ln)
                rec = sm_pool.tile([n, 512], F32, tag=f"rec{key}")
                nc.scalar.activation(rec, lns, EXP, scale=-1.0)
                dsc2 = dscr.tile([n, 512], F32, tag=f"dsc2{key}")
                nc.scalar.dma_start(out=dsc2, in_=rec)
                st.setdefault("recip_srcs", []).append((dsc2, r0, r1))

            # ---------------- projection work units ----------------
            def rbs_broadcast(st, src_idx, r0, r1):
                rbs = st["rbs"]
                dsc2, s0, _ = st["recip_srcs"][src_idx]
                for r in range(r0, r1):
                    row = dsc2[r - s0 : r - s0 + 1, :]
                    bcast = bass.AP(
                        tensor=row.tensor,
                        offset=row.offset,
                        ap=[[0, 64]] + [p for p in row.ap if p[1] != 1],
                    )
                    nc.scalar.dma_start(out=rbs[:, r, :], in_=bcast)

            def proj_units(bp, part, deep=False):
                """Work units (closures, ~0.2-0.8us PE each) for batch bp's
                normalize + projection, consumed one per attention block.
                part "a" = rbs rows 0-5 (qc0-2 denominators, bounced early at
                qc2) + chunks 0-2; part "b" = rbs rows 6-7 + chunk 3.
                deep=True (final tail only): alternate PSUM rings (ps1 +
                idle ps_s) and alternate DVE/ACT output casts so the drain
                runs dense instead of 2-buffer WAR-paced."""
                st = state[bp]
                units = []

                def u_rbs_a():
                    rbs = rb_pool.tile([64, 8, 512], F32, tag="rbs")
                    yT = yt_pool.tile([128, T], DT, tag="yT")
                    st["rbs"] = rbs
                    st["yT"] = yT
                    rbs_broadcast(st, 0, 0, 6)

                def u_rbs_b():
                    rbs_broadcast(st, 1, 6, 8)

                units.append(u_rbs_a if part == "a" else u_rbs_b)
                for k, tch in enumerate(range(3) if part == "a" else range(3, 4)):
                    for mt in range(8):

                        def u(tch=tch, mt=mt, i=len(units)):
                            yT = st["yT"]
                            if mt == 0:
                                yub, rbs = st["yub"], st["rbs"]
                                nc.vector.tensor_mul(
                                    yT[0:64, tch * 512 : (tch + 1) * 512],
                                    yub[0:64, 2 * tch, :],
                                    rbs[:, 2 * tch, :],
                                )
                                nc.vector.tensor_mul(
                                    yT[64:128, tch * 512 : (tch + 1) * 512],
                                    yub[0:64, 2 * tch + 1, :],
                                    rbs[:, 2 * tch + 1, :],
                                )
                            if deep and i % 2:
                                sdeep = ps_s.tile([128, 2, 512], F32, tag="s")
                                o = sdeep[:, 0, :]
                            else:
                                ops1 = ps1.tile([128, 512], F32, tag="ps1")
                                o = ops1
                            nc.tensor.matmul(
                                o,
                                lhsT=wp_sb[:, mt * 128 : (mt + 1) * 128],
                                rhs=yT[:, tch * 512 : (tch + 1) * 512],
                                start=True,
                                stop=True,
                            )
                            osb = ost_pool.tile([128, 512], BF16, tag="osb")
                            if deep and i % 2:
                                nc.scalar.copy(osb, o)
                            else:
                                nc.vector.tensor_copy(osb, o)
                            nc.sync.dma_start(
                                out=outT[
                                    mt * 128 : (mt + 1) * 128,
                                    bp * T + tch * 512 : bp * T + (tch + 1) * 512,
                                ],
                                in_=osb,
                            )

                        units.append(u)
                return units

            # ---------------- attention phase (pipelined + woven) ----------
            def emit_attention(b, filler, split_sums):
                st = state[b]
                qT, kT, va0, va1 = st["qT"], st["kT"], st["va0"], st["va1"]
                yub = yub_p.tile([65, 8, 512], F32, tag="yub")
                st["yub"] = yub
                ys = {}
                pq = []

                def flush(p):
                    qc, kb, pt, lo, last = p
                    if kb == 0:
                        # allocate at first write so the WAR against the
                        # previous qc's drain copies is sequenced correctly
                        y0 = ps_y.tile([65, 512], F32, tag="y0")
                        y1 = ps_y.tile([65, 512], F32, tag="y1")
                        ys[qc] = (y0, y1)
                    y0, y1 = ys[qc]
                    nkb = 4 * qc + 4
                    nc.tensor.matmul(
                        y0[:, lo:512],
                        lhsT=va0[:, kb, :],
                        rhs=pt[:, 0, lo:512],
                        start=(kb == 0),
                        stop=(kb == nkb - 1),
                    )
                    nc.tensor.matmul(
                        y1[:, lo:512],
                        lhsT=va1[:, kb, :],
                        rhs=pt[:, 1, lo:512],
                        start=(kb == 0),
                        stop=(kb == nkb - 1),
                    )
                    if last:
                        # stash y + denominator row; split across DVE/ACT
                        # (gpsimd can't read PSUM) so the single ps_y bank
                        # pair recycles fast
                        nc.vector.tensor_copy(yub[:, 2 * qc, :], y0[:, :])
                        nc.vector.tensor_copy(yub[:, 2 * qc + 1, :], y1[:, :])
                        if qc == 2:
                            emit_denom_bounce(b, 0, 6, "a")
                        if qc == 3:
                            emit_denom_bounce(b, 6, 8, "b")

                blocks = [(qc, kb) for qc in range(4) for kb in range(4 * qc + 4)]
                nblocks = len(blocks)
                for bi, (qc, kb) in enumerate(blocks):
                    diag = kb >= 4 * qc
                    lo = 128 * (kb - 4 * qc) if diag else 0
                    s = ps_s.tile([128, 2, 512], F32, tag="s")
                    nc.tensor.matmul(
                        s[:, 0, lo:512],
                        lhsT=kT[0:64, kb * 128 : (kb + 1) * 128],
                        rhs=qT[0:64, qc * 512 + lo : (qc + 1) * 512],
                        start=True,
                        stop=True,
                    )
                    nc.tensor.matmul(
                        s[:, 1, lo:512],
                        lhsT=kT[64:128, kb * 128 : (kb + 1) * 128],
                        rhs=qT[64:128, qc * 512 + lo : (qc + 1) * 512],
                        start=True,
                        stop=True,
                    )
                    pt = pt_pool.tile([128, 2, 512], DT, tag="pt")
                    nc.scalar.activation(pt[:, :, lo:512], s[:, :, lo:512], EXP, scale=0.125)
                    if diag:
                        nc.gpsimd.affine_select(
                            out=pt[:, :, lo : lo + 128],
                            in_=pt[:, :, lo : lo + 128],
                            pattern=[[0, 2], [1, 128]],
                            base=0,
                            channel_multiplier=-1,
                            compare_op=mybir.AluOpType.is_ge,
                            fill=0.0,
                        )
                    if len(pq) >= 2:
                        flush(pq.pop(0))
                    # adaptive pacing: drain the filler evenly over the
                    # remaining blocks (max 2 units per block)
                    remaining = nblocks - bi
                    npop = (2 if (split_sums and bi >= 26) else 1) if filler else 0
                    for _ in range(npop):
                        if filler:
                            filler.pop(0)()
                    pq.append((qc, kb, pt, lo, kb == 4 * qc + 3))
                while pq:
                    flush(pq.pop(0))

            # ---------------- batch schedule ----------------
            # batch 0's qkv runs dense up front; afterwards batch b's
            # attention weaves (a) batch b-1's projection and (b) batch b+1's
            # ENTIRE qkv as PE filler, so the ACT-bound attention phase and
            # the PE-only qkv phase fully overlap.
            emit_slab_dmas(0, 0, split_queues=True)
            emit_late_weights()
            emit_slab_dmas(0, 1)
            q0 = qkv_units(0)
            # dense prefix: u_init + the tch0 chunks + first transposes; the
            # rest of batch 0's qkv weaves into att(0) whose early blocks are
            # ACT-bound anyway
            prefix = [q0[0], q0[1], q0[3], q0[5], q0[2], q0[4], q0[6], q0[8]]
            rest = [u for u in q0 if u not in prefix]
            for u in prefix:
                u()
            for b in range(B):
                filler = []
                if b == 0:
                    filler.extend(rest)
                if b < B - 1:
                    filler.append(lambda bb=b + 1: emit_slab_dmas(bb, 0))
                if b > 0:
                    filler.extend(proj_units(b - 1, "a"))
                if b < B - 1:
                    filler.append(lambda bb=b + 1: emit_slab_dmas(bb, 1))
                if b > 0:
                    filler.extend(proj_units(b - 1, "b"))
                if b < B - 1:
                    filler.extend(qkv_units(b + 1))
                if b == B - 1:
                    filler.extend(proj_units(b, "a"))
                emit_attention(b, filler, split_sums=(b == B - 1))
                for u in filler:  # leftovers, in order
                    u()
            for u in proj_units(B - 1, "b", deep=True):
                u()

    _split_waits(nc)
    return nc


_nc_cache = None


def kernel(x, W_qkv, b_qkv, W_proj, b_proj):
    global _nc_cache
    import ml_dtypes

    x = np.ascontiguousarray(np.asarray(x, dtype=np.float32))
    W_qkv = np.asarray(W_qkv, dtype=np.float32)
    b_qkv = np.asarray(b_qkv, dtype=np.float32)
    W_proj = np.asarray(W_proj, dtype=np.float32)
    b_proj = np.asarray(b_proj, dtype=np.float32)

    npdt = mybir.dt.np(_op_dtype())
    xT = np.ascontiguousarray(x.reshape(B * T, C).T).astype(ml_dtypes.bfloat16)
    ident = np.eye(128, dtype=np.float32).astype(npdt)

    in_maps = []
    for i in range(NCORES):
        s = slice(128 * i, 128 * (i + 1))
        wq = np.ascontiguousarray(
            np.concatenate(
                [W_qkv[:, s], W_qkv[:, 1024:2048][:, s], W_qkv[:, 2048:3072][:, s]],
                axis=1,
            )
        ).astype(ml_dtypes.bfloat16)
        bq = np.ascontiguousarray(
            np.stack([b_qkv[0:1024][s], b_qkv[1024:2048][s], b_qkv[2048:3072][s]], axis=1)
        )
        wp = np.ascontiguousarray(W_proj[s, :]).astype(npdt)
        in_maps.append(
            {"xT": xT, "wqkv": wq, "bqkv": bq, "wproj": wp, "ident": ident}
        )

    if _nc_cache is None:
        _nc_cache = build_nc()
    res = run_bass_kernel_spmd(_nc_cache, in_maps, list(range(NCORES)), trace=TRACE)
    kernel.last_result = res

    acc = np.zeros((C, B * T), dtype=np.float32)
    for r in res.results:
        acc += np.asarray(r["outT"], dtype=np.float32)
    out = acc.T.reshape(B, T, C) + b_proj
    return out.astype(np.float32)


# revision 4
# speedup vs baseline: 1.0129x; 1.0129x over previous
"""Causal self-attention (B=4, T=2048, C=1024, 16 heads x d=64) on 8 trn2 NeuronCores.

Strategy: tensor-parallel over heads — core i owns heads (2i, 2i+1).
v2: engineered to keep the PE continuously busy at the full 2.4 GHz p-state
(TRN2 drops to 1.2 GHz whenever the tensor engine stalls, and only ramps back
after ~3us of continuous execution — the v1 trace showed nearly every matmul
running at the mid/low p-state):
  - attention k-loop is software-pipelined with depth 2: the P^T V matmuls of
    block i run during block i+2, so the PE never waits on the scalar-engine
    exp (or the gpsimd causal select on diagonal blocks)
  - the normalize+projection of batch b-1 is woven into batch b's attention
    loop as PE filler (1 matmul per block) covering the exp-rate deficit
  - batch 3 bounces its qc0-2 softmax denominators early so its own proj
    chunks 0-2 weave into its qc3 k-loop, shortening the tail
  - V-transpose copies + half the y drains run on gpsimd, so PSUM recycling
    never waits on the DVE
  - x / W_qkv are bf16 (same 1 cyc/row PE rate, half the slab DMA bytes);
    q/k/v/attention stay fp32r
  - S matmuls are trimmed to the causally reachable columns on diagonal blocks
  - denominator DMA bounces ride the scalar DGE queue so their semaphore waits
    never head-of-line-block slab/output traffic on the sync queue
Layout is feature-major as in v1: host pre-transposes x once; per-core output
is the 128-feature partial projection out^T [C, B*T], summed on host.
"""

import sys

if "/opt/trn_rl_repo" not in sys.path:
    sys.path.insert(0, "/opt/trn_rl_repo")

import contextlib
import ctypes
import types

import numpy as np

import concourse.bass as bass
import concourse.mybir as mybir
import concourse.tile as tile
from concourse.bass_utils import run_bass_kernel_spmd

B, T, C = 4, 2048, 1024
N_HEAD, D = 16, 64
NCORES = 8
F32 = mybir.dt.float32
BF16 = mybir.dt.bfloat16

# matmul operand dtype for attention/proj tensors: "fp32r" (1 cyc/row at
# free-dim >= 256) or "fp32" (bit-exact, 4 cyc/row). x/W_qkv are always bf16.
DT_MM = "fp32r"
TRACE = False  # test.py flips this for profiled runs

_SO_PATH = "/opt/axon/libaxon_pjrt.so"


# ---------------------------------------------------------------------------
# Environment shims: (1) register the NTFF profile hook trn_boot could not
# install (image's antenv lacks axon_hooks); (2) this walrus build caps sem
# waits per instruction, but Tile's tail drain carries one wait per active
# proc — spread them over single-wait SP NOPs instead.
# ---------------------------------------------------------------------------
def _install_ntff_hook():
    if "antenv.axon_hooks" in sys.modules:
        return
    state = {"hook": None}

    def set_hook(h):
        state["hook"] = h

    def get_hook():
        return state["hook"]

    mod = types.ModuleType("antenv.axon_hooks")
    mod.set_axon_ntff_profile_hook = set_hook
    mod.get_axon_ntff_profile_hook = get_hook
    sys.modules["antenv.axon_hooks"] = mod
    import antenv

    antenv.axon_hooks = mod

    try:
        lib = ctypes.CDLL(_SO_PATH)
    except OSError:
        return
    if not hasattr(lib, "axon_start_nrt_profile"):
        return
    lib.axon_start_nrt_profile.argtypes = [
        ctypes.POINTER(ctypes.c_int64),
        ctypes.c_size_t,
    ]
    lib.axon_start_nrt_profile.restype = ctypes.c_int64
    lib.axon_stop_nrt_profile.argtypes = [ctypes.c_char_p]
    lib.axon_stop_nrt_profile.restype = ctypes.c_int64

    @contextlib.contextmanager
    def _hook_cm(output_dir, device_ids):
        import jax

        jax.devices()
        if device_ids:
            ids = (ctypes.c_int64 * len(device_ids))(*device_ids)
            rc = lib.axon_start_nrt_profile(ids, len(device_ids))
        else:
            rc = lib.axon_start_nrt_profile(None, 0)
        if rc != 0:
            raise RuntimeError(f"axon_start_nrt_profile rc={rc}")
        try:
            yield
        finally:
            n = lib.axon_stop_nrt_profile(str(output_dir).encode())
            if n < 0:
                raise RuntimeError(f"axon_stop_nrt_profile rc={n}")
            print(f"profile: {n} file(s) written to {output_dir}", file=sys.stderr)

    set_hook(_hook_cm)


def _patch_tile_tail_drain():
    from concourse.vector_clock import ScopedClock, VectorClock

    if getattr(tile.TileContext, "_drain_patch", False):
        return

    def patched(self, tick_clock, wait_clock):
        vc = tick_clock.global_clock
        n = len(vc)
        for proc in range(n):
            t = vc[proc]
            if t <= 0:
                continue
            sub = VectorClock([t if i == proc else 0 for i in range(n)])
            nop = self.nc.sync.nop(nofuse=True)
            wait_clock.add_sem_waits(nop.ins, ScopedClock({None: sub}))
        # Same tail as the original _drain_and_barrier, minus the multi-wait
        # drain — the NOP chain above already waited on every proc.
        self.nc.sync.drain()
        self.nc.all_engine_barrier()
        assert self.sems is not None
        popped = self.nc._tile_sem_poison_stack.pop()
        assert popped is self._sem_poison
        self.nc.clear_and_free_semaphores(list(self.sems.allocated().values()))
        self.nc.all_engine_barrier()

    tile.TileContext._drain_and_barrier = patched
    tile.TileContext._drain_patch = True


_install_ntff_hook()
_patch_tile_tail_drain()


def _split_waits(nc, limit=1):
    """This walrus build rejects instructions carrying more than ~2 sem waits.
    Spill excess waits onto preceding same-engine NOPs (program order on the
    issuing engine preserves the blocking semantics exactly)."""
    k = 0
    for fn in nc.m.functions:
        for bb in fn.blocks:
            new = []
            for ins in bb.instructions:
                si = ins.sync_info
                waits = list(si.on_wait) if si and si.on_wait else []
                if len(waits) > limit:
                    for w in waits[:-limit]:
                        nop = mybir.InstNoOp(name=f"I-wsplit-{k}")
                        k += 1
                        nop.engine = ins.engine
                        nop.sync_info = mybir.SyncInfo(on_wait=[w], on_update=[])
                        new.append(nop)
                    ins.sync_info = mybir.SyncInfo(
                        on_wait=waits[-limit:],
                        on_update=list(si.on_update) if si.on_update else [],
                    )
                new.append(ins)
            bb.instructions = new


def _op_dtype():
    return {
        "fp32": mybir.dt.float32,
        "fp32r": mybir.dt.float32r,
        "bf16": mybir.dt.bfloat16,
    }[DT_MM]


def build_nc():
    DT = _op_dtype()
    nc = bass.Bass()
    xT = nc.declare_dram_parameter("xT", [C, B * T], BF16, isOutput=False)
    wqkv = nc.declare_dram_parameter("wqkv", [C, 384], BF16, isOutput=False)
    bqkv = nc.declare_dram_parameter("bqkv", [128, 3], F32, isOutput=False)
    wproj = nc.declare_dram_parameter("wproj", [128, C], DT, isOutput=False)
    ident = nc.declare_dram_parameter("ident", [128, 128], DT, isOutput=False)
    outT = nc.declare_dram_parameter("outT", [C, B * T], BF16, isOutput=True)

    EXP = mybir.ActivationFunctionType.Exp

    wide = mybir.dt.size(DT) > 2  # fp32 debug dtype needs smaller pools
    with tile.TileContext(nc) as tc:
        with contextlib.ExitStack() as ctx:
            singles = ctx.enter_context(tc.tile_pool(name="singles", bufs=1))
            xpool = ctx.enter_context(tc.tile_pool(name="xpool", bufs=16))
            qkv_sb = ctx.enter_context(tc.tile_pool(name="qkv_sb", bufs=2))
            vt_pool = ctx.enter_context(tc.tile_pool(name="vtp", bufs=1))
            vaug_p = ctx.enter_context(tc.tile_pool(name="vaug", bufs=2))
            pt_pool = ctx.enter_context(tc.tile_pool(name="ptp", bufs=2 if wide else 4))
            yt_pool = ctx.enter_context(tc.tile_pool(name="ytp", bufs=1 if wide else 2))
            yub_p = ctx.enter_context(tc.tile_pool(name="yub", bufs=1 if wide else 2))
            sm_pool = ctx.enter_context(tc.tile_pool(name="smp", bufs=1))
            rb_pool = ctx.enter_context(tc.tile_pool(name="rbp", bufs=1 if wide else 2))
            ost_pool = ctx.enter_context(tc.tile_pool(name="ost", bufs=3 if wide else 6))
            dscr = ctx.enter_context(tc.tile_pool(name="dscr", bufs=2, space="DRAM"))
            # PSUM (8 banks): s [128,2,512] x2 = 4, y0+y1 [65,512] x1 = 2,
            # ps1 [128,512] x2 = 2 (qkv accum / transposes / proj out)
            ps1 = ctx.enter_context(tc.tile_pool(name="ps1", bufs=2, space="PSUM"))
            ps_s = ctx.enter_context(tc.tile_pool(name="ps_s", bufs=2, space="PSUM"))
            ps_y = ctx.enter_context(tc.tile_pool(name="ps_y", bufs=1, space="PSUM"))

            # weights ride the scalar DGE queue so the sync queue starts on
            # x slabs immediately
            wq_sb = singles.tile([128, 8, 384], BF16)
            nc.scalar.dma_start(out=wq_sb, in_=wqkv.rearrange("(a p) f -> p a f", p=128))
            wp_sb = singles.tile([128, C], DT)
            bq_sb = singles.tile([128, 3], F32)
            id_sb = singles.tile([128, 128], DT)

            def emit_late_weights():
                # bq/id are first needed ~12us in, wp only by the first
                # projection — don't let them delay the slab prefetch
                nc.scalar.dma_start(out=bq_sb, in_=bqkv[:, :])
                nc.scalar.dma_start(out=id_sb, in_=ident[:, :])
                nc.scalar.dma_start(out=wp_sb, in_=wproj[:, :])
            ones_col = singles.tile([128, 16, 1], F32)
            nc.vector.memset(ones_col, 1.0)
            # preload the Exp act table during the first qkv phase
            warm = singles.tile([1, 1], F32)
            nc.scalar.activation(warm, ones_col[0:1, 0, 0:1], EXP, scale=1.0)

            state = {}

            # ---------------- qkv phase (dense) ----------------
            def emit_slab_dmas(b, g, split_queues=False):
                st = state.setdefault(b, {"slabs": {}})
                slabs = []
                for c in range(8):
                    sl = xpool.tile([128, 1024], BF16, tag="xslab")
                    eng = nc.scalar if (split_queues and c % 2) else nc.sync
                    eng.dma_start(
                        out=sl,
                        in_=xT[
                            c * 128 : (c + 1) * 128,
                            b * T + g * 1024 : b * T + (g + 1) * 1024,
                        ],
                    )
                    slabs.append(sl)
                st["slabs"][g] = slabs

            def qkv_units(b):
                """qkv work as a list of ~0.4-1.9us PE closures so it can
                weave into the previous batch's ACT-bound attention loop:
                12 chunk units (8 accumulation matmuls + DVE bias-add) and
                8 transpose units (2 V transposes + DVE va drains)."""
                st = state.setdefault(b, {"slabs": {}})

                def u_init():
                    qT = qkv_sb.tile([128, T], DT, tag="qT")
                    kT = qkv_sb.tile([128, T], DT, tag="kT")
                    vT = vt_pool.tile([128, T], DT, tag="vT")
                    va0 = vaug_p.tile([128, 16, 65], DT, tag="va0")
                    va1 = vaug_p.tile([128, 16, 65], DT, tag="va1")
                    nc.gpsimd.tensor_copy(va0[:, :, 64:65], ones_col)
                    nc.gpsimd.tensor_copy(va1[:, :, 64:65], ones_col)
                    st.update(qT=qT, kT=kT, vT=vT, va0=va0, va1=va1)

                def chunk(g, m, half):
                    def u():
                        dst = (st["qT"], st["kT"], st["vT"])[m]
                        slabs = st["slabs"][g]
                        tch = g * 2 + half
                        ps = ps1.tile([128, 512], F32, tag="ps1")
                        for c in range(8):
                            nc.tensor.matmul(
                                ps,
                                lhsT=wq_sb[:, c, m * 128 : (m + 1) * 128],
                                rhs=slabs[c][:, half * 512 : (half + 1) * 512],
                                start=(c == 0),
                                stop=(c == 7),
                            )
                        nc.vector.tensor_scalar_add(
                            dst[:, tch * 512 : (tch + 1) * 512], ps, bq_sb[:, m : m + 1]
                        )

                    return u

                def transp2(tt0):
                    def u():
                        for tt in (tt0, tt0 + 1):
                            tp = ps1.tile([128, 128], DT, tag="ps1")
                            nc.tensor.transpose(
                                tp, st["vT"][:, tt * 128 : (tt + 1) * 128], id_sb
                            )
                            nc.vector.tensor_copy(st["va0"][:, tt, 0:64], tp[:, 0:64])
                            nc.vector.tensor_copy(st["va1"][:, tt, 0:64], tp[:, 64:128])

                    return u

                units = [u_init]
                for m in range(3):
                    for half in range(2):
                        units.append(chunk(0, m, half))
                tts = [0, 2, 4, 6]
                for m in range(3):
                    for half in range(2):
                        units.append(chunk(1, m, half))
                        if tts:
                            units.append(transp2(tts.pop(0)))
                for tt0 in (8, 10, 12, 14):
                    units.append(transp2(tt0))
                return units

            # ---------------- denominator bounce ----------------
            def emit_denom_bounce(b, r0, r1, key):
                """DMA yub partition-64 rows [r0,r1) to DRAM, back as
                [r1-r0, 512] (re-partition), fast-approx reciprocal on DVE
                (denominators are sums of positive exps — far from the
                undefined edge cases), back out to DRAM for the later
                partition-broadcast. All DMAs ride the scalar DGE queue so
                their waits never head-of-line-block slab/output traffic on
                the sync queue."""
                st = state[b]
                n = r1 - r0
                dsc = dscr.tile([1, n, 512], F32, tag=f"dsc{key}")
                nc.scalar.dma_start(out=dsc, in_=st["yub"][64:65, r0:r1, :])
                sums = sm_pool.tile([n, 512], F32, tag=f"sums{key}")
                nc.scalar.dma_start(out=sums, in_=dsc.rearrange("o h q -> (o h) q"))
                # 1/x = exp(-ln x) on the ACT engine: Ln/Exp/Copy share one
                # act table set, so no table reloads and no DVE coupling
                lns = sm_pool.tile([n, 512], F32, tag=f"lns{key}")
                nc.scalar.activation(lns, sums, mybir.ActivationFunctionType.Ln)
                rec = sm_pool.tile([n, 512], F32, tag=f"rec{key}")
                nc.scalar.activation(rec, lns, EXP, scale=-1.0)
                dsc2 = dscr.tile([n, 512], F32, tag=f"dsc2{key}")
                nc.scalar.dma_start(out=dsc2, in_=rec)
                st.setdefault("recip_srcs", []).append((dsc2, r0, r1))

            # ---------------- projection work units ----------------
            def rbs_broadcast(st, src_idx, r0, r1):
                rbs = st["rbs"]
                dsc2, s0, _ = st["recip_srcs"][src_idx]
                for r in range(r0, r1):
                    row = dsc2[r - s0 : r - s0 + 1, :]
                    bcast = bass.AP(
                        tensor=row.tensor,
                        offset=row.offset,
                        ap=[[0, 64]] + [p for p in row.ap if p[1] != 1],
                    )
                    nc.scalar.dma_start(out=rbs[:, r, :], in_=bcast)

            def proj_units(bp, part, deep=False):
                """Work units (closures, ~0.2-0.8us PE each) for batch bp's
                normalize + projection, consumed one per attention block.
                part "a" = rbs rows 0-5 (qc0-2 denominators, bounced early at
                qc2) + chunks 0-2; part "b" = rbs rows 6-7 + chunk 3.
                deep=True (final tail only): alternate PSUM rings (ps1 +
                idle ps_s) and alternate DVE/ACT output casts so the drain
                runs dense instead of 2-buffer WAR-paced."""
                st = state[bp]
                units = []

                def u_rbs_a():
                    rbs = rb_pool.tile([64, 8, 512], F32, tag="rbs")
                    yT = yt_pool.tile([128, T], DT, tag="yT")
                    st["rbs"] = rbs
                    st["yT"] = yT
                    rbs_broadcast(st, 0, 0, 6)

                def u_rbs_b():
                    rbs_broadcast(st, 1, 6, 8)

                units.append(u_rbs_a if part == "a" else u_rbs_b)
                for k, tch in enumerate(range(3) if part == "a" else range(3, 4)):
                    for mt in range(8):

                        def u(tch=tch, mt=mt, i=len(units)):
                            yT = st["yT"]
                            if mt == 0:
                                yub, rbs = st["yub"], st["rbs"]
                                nc.vector.tensor_mul(
                                    yT[0:64, tch * 512 : (tch + 1) * 512],
                                    yub[0:64, 2 * tch, :],
                                    rbs[:, 2 * tch, :],
                                )
                                nc.vector.tensor_mul(
                                    yT[64:128, tch * 512 : (tch + 1) * 512],
                                    yub[0:64, 2 * tch + 1, :],
                                    rbs[:, 2 * tch + 1, :],
                                )
                            if deep and i % 2:
                                sdeep = ps_s.tile([128, 2, 512], F32, tag="s")
                                o = sdeep[:, 0, :]
                            else:
                                ops1 = ps1.tile([128, 512], F32, tag="ps1")
                                o = ops1
                            nc.tensor.matmul(
                                o,
                                lhsT=wp_sb[:, mt * 128 : (mt + 1) * 128],
                                rhs=yT[:, tch * 512 : (tch + 1) * 512],
                                start=True,
                                stop=True,
                            )
                            osb = ost_pool.tile([128, 512], BF16, tag="osb")
                            if deep and i % 2:
                                nc.scalar.copy(osb, o)
                            else:
                                nc.vector.tensor_copy(osb, o)
                            nc.sync.dma_start(
                                out=outT[
                                    mt * 128 : (mt + 1) * 128,
                                    bp * T + tch * 512 : bp * T + (tch + 1) * 512,
                                ],
                                in_=osb,
                            )

                        units.append(u)
                return units

            # ---------------- attention phase (pipelined + woven) ----------
            def emit_attention(b, filler, split_sums):
                st = state[b]
                qT, kT, va0, va1 = st["qT"], st["kT"], st["va0"], st["va1"]
                yub = yub_p.tile([65, 8, 512], F32, tag="yub")
                st["yub"] = yub
                ys = {}
                pq = []

                def flush(p):
                    qc, kb, pt, lo, last = p
                    if kb == 0:
                        # allocate at first write so the WAR against the
                        # previous qc's drain copies is sequenced correctly
                        y0 = ps_y.tile([65, 512], F32, tag="y0")
                        y1 = ps_y.tile([65, 512], F32, tag="y1")
                        ys[qc] = (y0, y1)
                    y0, y1 = ys[qc]
                    nkb = 4 * qc + 4
                    nc.tensor.matmul(
                        y0[:, lo:512],
                        lhsT=va0[:, kb, :],
                        rhs=pt[:, 0, lo:512],
                        start=(kb == 0),
                        stop=(kb == nkb - 1),
                    )
                    nc.tensor.matmul(
                        y1[:, lo:512],
                        lhsT=va1[:, kb, :],
                        rhs=pt[:, 1, lo:512],
                        start=(kb == 0),
                        stop=(kb == nkb - 1),
                    )
                    if last:
                        # stash y + denominator row; split across DVE/ACT
                        # (gpsimd can't read PSUM) so the single ps_y bank
                        # pair recycles fast
                        nc.vector.tensor_copy(yub[:, 2 * qc, :], y0[:, :])
                        nc.vector.tensor_copy(yub[:, 2 * qc + 1, :], y1[:, :])
                        if qc == 2:
                            emit_denom_bounce(b, 0, 6, "a")
                        if qc == 3:
                            emit_denom_bounce(b, 6, 8, "b")

                blocks = [(qc, kb) for qc in range(4) for kb in range(4 * qc + 4)]
                nblocks = len(blocks)
                for bi, (qc, kb) in enumerate(blocks):
                    diag = kb >= 4 * qc
                    lo = 128 * (kb - 4 * qc) if diag else 0
                    s = ps_s.tile([128, 2, 512], F32, tag="s")
                    nc.tensor.matmul(
                        s[:, 0, lo:512],
                        lhsT=kT[0:64, kb * 128 : (kb + 1) * 128],
                        rhs=qT[0:64, qc * 512 + lo : (qc + 1) * 512],
                        start=True,
                        stop=True,
                    )
                    nc.tensor.matmul(
                        s[:, 1, lo:512],
                        lhsT=kT[64:128, kb * 128 : (kb + 1) * 128],
                        rhs=qT[64:128, qc * 512 + lo : (qc + 1) * 512],
                        start=True,
                        stop=True,
                    )
                    pt = pt_pool.tile([128, 2, 512], DT, tag="pt")
                    nc.scalar.activation(pt[:, :, lo:512], s[:, :, lo:512], EXP, scale=0.125)
                    if diag:
                        nc.gpsimd.affine_select(
                            out=pt[:, :, lo : lo + 128],
                            in_=pt[:, :, lo : lo + 128],
                            pattern=[[0, 2], [1, 128]],
                            base=0,
                            channel_multiplier=-1,
                            compare_op=mybir.AluOpType.is_ge,
                            fill=0.0,
                        )
                    if len(pq) >= 2:
                        flush(pq.pop(0))
                    # adaptive pacing: drain the filler evenly over the
                    # remaining blocks (max 2 units per block)
                    remaining = nblocks - bi
                    npop = (2 if (split_sums and bi >= 30) else 1) if filler else 0
                    for _ in range(npop):
                        if filler:
                            filler.pop(0)()
                    pq.append((qc, kb, pt, lo, kb == 4 * qc + 3))
                while pq:
                    flush(pq.pop(0))

            # ---------------- batch schedule ----------------
            # batch 0's qkv runs dense up front; afterwards batch b's
            # attention weaves (a) batch b-1's projection and (b) batch b+1's
            # ENTIRE qkv as PE filler, so the ACT-bound attention phase and
            # the PE-only qkv phase fully overlap.
            emit_slab_dmas(0, 0, split_queues=True)
            emit_late_weights()
            emit_slab_dmas(0, 1)
            q0 = qkv_units(0)
            # dense prefix: u_init + the tch0 chunks + first transposes; the
            # rest of batch 0's qkv weaves into att(0) whose early blocks are
            # ACT-bound anyway
            prefix = [q0[0], q0[1], q0[3], q0[5], q0[2], q0[4], q0[6], q0[8]]
            rest = [u for u in q0 if u not in prefix]
            for u in prefix:
                u()
            for b in range(B):
                filler = []
                if b == 0:
                    filler.extend(rest)
                if b < B - 1:
                    filler.append(lambda bb=b + 1: emit_slab_dmas(bb, 0))
                if b > 0:
                    filler.extend(proj_units(b - 1, "a"))
                if b < B - 1:
                    filler.append(lambda bb=b + 1: emit_slab_dmas(bb, 1))
                if b > 0:
                    filler.extend(proj_units(b - 1, "b"))
                if b < B - 1:
                    filler.extend(qkv_units(b + 1))
                if b == B - 1:
                    filler.extend(proj_units(b, "a"))
                emit_attention(b, filler, split_sums=(b == B - 1))
                for u in filler:  # leftovers, in order
                    u()
            for u in proj_units(B - 1, "b", deep=True):
                u()

    _split_waits(nc)
    return nc


_nc_cache = None


def kernel(x, W_qkv, b_qkv, W_proj, b_proj):
    global _nc_cache
    import ml_dtypes

    x = np.ascontiguousarray(np.asarray(x, dtype=np.float32))
    W_qkv = np.asarray(W_qkv, dtype=np.float32)
    b_qkv = np.asarray(b_qkv, dtype=np.float32)
    W_proj = np.asarray(W_proj, dtype=np.float32)
    b_proj = np.asarray(b_proj, dtype=np.float32)

    npdt = mybir.dt.np(_op_dtype())
    xT = np.ascontiguousarray(x.reshape(B * T, C).T).astype(ml_dtypes.bfloat16)
    ident = np.eye(128, dtype=np.float32).astype(npdt)

    in_maps = []
    for i in range(NCORES):
        s = slice(128 * i, 128 * (i + 1))
        wq = np.ascontiguousarray(
            np.concatenate(
                [W_qkv[:, s], W_qkv[:, 1024:2048][:, s], W_qkv[:, 2048:3072][:, s]],
                axis=1,
            )
        ).astype(ml_dtypes.bfloat16)
        bq = np.ascontiguousarray(
            np.stack([b_qkv[0:1024][s], b_qkv[1024:2048][s], b_qkv[2048:3072][s]], axis=1)
        )
        wp = np.ascontiguousarray(W_proj[s, :]).astype(npdt)
        in_maps.append(
            {"xT": xT, "wqkv": wq, "bqkv": bq, "wproj": wp, "ident": ident}
        )

    if _nc_cache is None:
        _nc_cache = build_nc()
    res = run_bass_kernel_spmd(_nc_cache, in_maps, list(range(NCORES)), trace=TRACE)
    kernel.last_result = res

    acc = np.zeros((C, B * T), dtype=np.float32)
    for r in res.results:
        acc += np.asarray(r["outT"], dtype=np.float32)
    out = acc.T.reshape(B, T, C) + b_proj
    return out.astype(np.float32)
